# revision 24
# baseline (speedup 1.0000x reference)
"""Contrastive loss (SimCLR-style semi_loss pair) on 8 Trainium2 NeuronCores.

Math (reference):
    z1n, z2n = L2-normalized rows of z1, z2            # [N, D], N=16384, D=128
    S11 = z1n @ z1n.T, S12 = z1n @ z2n.T, S22 = z2n @ z2n.T, S21 = S12.T
    d1_i = sum_j exp(2*S11_ij) - exp(2*S11_ii) + sum_j exp(2*S12_ij)
    d2_i = sum_j exp(2*S22_ij) - exp(2*S22_ii) + sum_j exp(2*S21_ij)
    loss = mean_i( 0.5*(log d1_i + log d2_i) - 2*S12_ii )

Device strategy (row-parallel over N, 8 cores, D=128 on the partition axis
so every Gram tile is one K=128 matmul):

* Every exponential is computed exactly once (ScalarE), on wide PSUM spans
  (2048/1536 alternating between two sim slots, 7 banks). Row sums ride
  the activation accumulator; exp values are also written to SBUF (bf16).
* Column sums (exp(2*S21) row sums; mirror halves of the symmetric refl
  matrices) use selector-weight matmuls: weights with ones in column j
  only, so the matmul adds the 512-wide column sum into row j of ONE
  shared PSUM accumulator bank and exact zeros everywhere else. All 32
  column chunks of a stream accumulate in that single bank (the 8th).
* S11/S22 are symmetric: only spans >= the 1024-aligned diagonal square
  are computed (row chunk g covers columns [1024g, 16384)); the lower
  triangle of each row sum is recovered from the column sums, with the
  diagonal square excluded from column sums to avoid double counting.
* SPMD: one NEFF for all 8 cores. Rows are sharded STRIDED (core c owns
  row chunks {128*(8g+c)}, g=0..15) so all cores share the same
  column-range structure. Host packs row blocks into `zb`, builds the
  selector weights, and does the final O(N) combine (column-sum
  all-reduce, logs, mean).
"""

import os

import numpy as np

N = 16384
D = 128
NCORES = 8
B = N // NCORES  # 2048 rows per core
TAU = 0.5
SCALE = 1.0 / TAU
EPS = 1e-12

G = 16  # row-chunk groups per core (128 rows each); chunk g -> cols >= 1024g
NCH = N // 512  # 32 column chunks of 512 (cs accumulator rows)

WA = 2048  # sim slot A width (4 banks)
WB = 1536  # sim slot B width (3 banks)


def _spans(l0, use_a=True):
    """Alternating A/B spans (last may be partial) covering [l0*512, N)."""
    spans = []
    off = l0 * 512
    while off < N:
        w = min(WA if use_a else WB, N - off)
        spans.append((off, w, use_a))
        use_a = not use_a
        off += w
    return spans, use_a


def _stream_plan(tri):
    """Per-chunk span lists with A/B alternation carried across chunks."""
    plan = []
    use_a = True
    for g in range(G):
        spans, use_a = _spans(2 * g if tri else 0, use_a)
        plan.append(spans)
    return plan


_cache = {}


def _build():
    from contextlib import ExitStack

    import concourse.mybir as mybir
    from concourse import bacc
    from concourse.tile import TileContext

    f32 = mybir.dt.float32
    bf16 = mybir.dt.bfloat16
    Exp = mybir.ActivationFunctionType.Exp
    add = mybir.AluOpType.add
    AX = mybir.AxisListType.X

    # Bacc (vs plain Bass) runs the wait-legalization passes at finalize:
    # move_matmul_waits_to_ldweights + generate_event_semaphores (TRN2 allows
    # at most one sync wait per hardware instruction).
    nc = bacc.Bacc(None, target_bir_lowering=False, name="contrastive_loss")

    z1t = nc.declare_dram_parameter("z1t", [D, N], bf16, isOutput=False)
    z2t = nc.declare_dram_parameter("z2t", [D, N], bf16, isOutput=False)
    # per-core strided row chunks: [z1 chunks g=0..15 | z2 chunks g=0..15]
    zb = nc.declare_dram_parameter("zb", [D, 2 * B], bf16, isOutput=False)
    # selector weights: sel[:, 128j + m] = 1 if m == j else 0 (j = 0..31)
    sel = nc.declare_dram_parameter("sel", [D, NCH * 128], bf16, isOutput=False)

    rs11_d = nc.declare_dram_parameter("rs11", [128, G], f32, isOutput=True)
    rs22_d = nc.declare_dram_parameter("rs22", [128, G], f32, isOutput=True)
    rs12_d = nc.declare_dram_parameter("rs12", [128, G], f32, isOutput=True)
    cs11_d = nc.declare_dram_parameter("cs11", [NCH, 512], f32, isOutput=True)
    cs22_d = nc.declare_dram_parameter("cs22", [NCH, 512], f32, isOutput=True)
    cs12_d = nc.declare_dram_parameter("cs12", [NCH, 512], f32, isOutput=True)
    # raw diagonal dot products: pairs (z1,z1),(z1,z2),(z2,z2), 4 chunks each
    dg_d = nc.declare_dram_parameter("diags", [12, 512], f32, isOutput=True)

    with TileContext(nc) as tc, ExitStack() as ctx:
        const = ctx.enter_context(tc.tile_pool(name="const", bufs=1))
        prodp = ctx.enter_context(tc.tile_pool(name="prodp", bufs=3))
        outp = ctx.enter_context(tc.tile_pool(name="outp", bufs=1))
        esbp = ctx.enter_context(tc.tile_pool(name="esbp", bufs=2))
        csout = ctx.enter_context(tc.tile_pool(name="csout", bufs=2))
        psS = ctx.enter_context(tc.tile_pool(name="psS", bufs=1, space="PSUM"))
        psCS = ctx.enter_context(tc.tile_pool(name="psCS", bufs=1, space="PSUM"))

        zb_sb = const.tile([128, 2 * B], bf16)
        sel_sb = const.tile([128, NCH * 128], bf16)
        z1t_sb = const.tile([128, N], bf16)
        z2t_sb = const.tile([128, N], bf16)
        nc.sync.dma_start(out=zb_sb, in_=zb[:, :])
        nc.sync.dma_start(out=sel_sb, in_=sel[:, :])
        ncol = N // 4
        for i in range(4):
            nc.sync.dma_start(
                out=z1t_sb[:, i * ncol : (i + 1) * ncol],
                in_=z1t[:, i * ncol : (i + 1) * ncol],
            )
        for i in range(4):
            nc.sync.dma_start(
                out=z2t_sb[:, i * ncol : (i + 1) * ncol],
                in_=z2t[:, i * ncol : (i + 1) * ncol],
            )
        z1b_sb = zb_sb[:, 0:B]
        z2b_sb = zb_sb[:, B : 2 * B]

        rs11_sb = outp.tile([128, G], f32, tag="rs11")
        rs22_sb = outp.tile([128, G], f32, tag="rs22")
        rs12_sb = outp.tile([128, G], f32, tag="rs12")
        parts11 = outp.tile([128, G * 16], f32, tag="p11")
        parts22 = outp.tile([128, G * 16], f32, tag="p22")
        parts12 = outp.tile([128, G * 16], f32, tag="p12")

        def wsel(j):
            return sel_sb[:, j * 128 : (j + 1) * 128]

        # ---- Phase 0: raw diagonals diag[i] = sum_d a[d,i]*b[d,i] ----
        # elementwise products, then selector-matmul column sums into one
        # accumulator bank (rows 0..11), evacuated once.
        dgps = psCS.tile([128, 512], f32, tag="cs", name="dgps")
        pairs = [(z1b_sb, z1b_sb), (z1b_sb, z2b_sb), (z2b_sb, z2b_sb)]
        first = True
        for di, (a, b) in enumerate(pairs):
            prod = prodp.tile([128, B], bf16)
            nc.vector.tensor_mul(prod, a, b)
            for k in range(4):
                nc.tensor.matmul(
                    dgps,
                    lhsT=wsel(4 * di + k),
                    rhs=prod[:, k * 512 : (k + 1) * 512],
                    start=first,
                    stop=(di == 2 and k == 3),
                )
                first = False
        dg_sb = csout.tile([12, 512], f32, tag="dg")
        nc.vector.tensor_copy(out=dg_sb, in_=dgps[0:12, :])
        nc.sync.dma_start(out=dg_d[:, :], in_=dg_sb)

        # ---- main streams ----
        # (zoff, rhs, tri, parts, cs_d, rs_sb, rs_d)
        streams = [
            (0, z1t_sb, True, parts11, cs11_d, rs11_sb, rs11_d),
            (B, z2t_sb, True, parts22, cs22_d, rs22_sb, rs22_d),
            (0, z2t_sb, False, parts12, cs12_d, rs12_sb, rs12_d),
        ]
        for si, (zoff, full, tri, parts, cs_d_, rs_sb, rs_d) in enumerate(streams):
            plan = _stream_plan(tri)
            # enumerate the column-sum matmuls up front so start/stop flags
            # land on the stream's true first/last ones
            n_ones = sum(
                1
                for g in range(G)
                for (off, width, _a) in plan[g]
                for k in range(width // 512)
                if not (tri and (off + k * 512) // 512 in (2 * g, 2 * g + 1))
            )
            csps = psCS.tile([128, 512], f32, tag="cs", name=f"csps{si}")
            ones_idx = 0
            for g in range(G):
                for sp, (off, width, use_a) in enumerate(plan[g]):
                    nk = width // 512
                    sim = psS.tile(
                        [128, WA if use_a else WB],
                        f32,
                        tag="simA" if use_a else "simB",
                        name="simA_t" if use_a else "simB_t",
                    )
                    for k in range(nk):
                        col = off + k * 512
                        nc.tensor.matmul(
                            sim[:, k * 512 : (k + 1) * 512],
                            lhsT=zb_sb[:, zoff + g * 128 : zoff + (g + 1) * 128],
                            rhs=full[:, col : col + 512],
                            start=True,
                            stop=True,
                        )
                    esb = esbp.tile(
                        [128, WA if use_a else WB],
                        bf16,
                        tag="esbA" if use_a else "esbB",
                        name="esbA_t" if use_a else "esbB_t",
                    )
                    nc.scalar.activation(
                        out=esb[:, 0:width],
                        in_=sim[:, 0:width],
                        func=Exp,
                        scale=SCALE,
                        accum_out=parts[:, g * 16 + sp : g * 16 + sp + 1],
                    )
                    for k in range(nk):
                        j = (off + k * 512) // 512
                        if tri and j in (2 * g, 2 * g + 1):
                            continue
                        nc.tensor.matmul(
                            csps,
                            lhsT=wsel(j),
                            rhs=esb[:, k * 512 : (k + 1) * 512],
                            start=(ones_idx == 0),
                            stop=(ones_idx == n_ones - 1),
                        )
                        ones_idx += 1
            cs_sb = csout.tile([NCH, 512], f32, tag="cs", name=f"cs_sb{si}")
            nc.vector.tensor_copy(out=cs_sb, in_=csps[0:NCH, :])
            nc.sync.dma_start(out=cs_d_[:, :], in_=cs_sb)

            # row sums: reduce the per-span partials
            for g in range(G):
                nsp = len(plan[g])
                nc.vector.tensor_reduce(
                    out=rs_sb[:, g : g + 1],
                    in_=parts[:, g * 16 : g * 16 + nsp],
                    axis=AX,
                    op=add,
                )
            nc.sync.dma_start(out=rs_d[:, :], in_=rs_sb)

    nc.finalize()  # Bacc: runs wait-legalization + register allocation
    return nc


def _get_nc():
    if "nc" not in _cache:
        _cache["nc"] = _build()
    return _cache["nc"]


def _sel_weights():
    import ml_dtypes

    w = np.zeros((D, NCH, 128), dtype=np.float32)
    for j in range(NCH):
        w[:, j, j] = 1.0
    return np.ascontiguousarray(w.reshape(D, NCH * 128)).astype(ml_dtypes.bfloat16)


def kernel(z1: np.ndarray, z2: np.ndarray) -> np.ndarray:
    import ml_dtypes

    from concourse.bass_utils import run_bass_kernel_spmd

    z1 = np.asarray(z1, dtype=np.float32)
    z2 = np.asarray(z2, dtype=np.float32)

    # host: L2 row-normalize (matches F.normalize eps clamp), transpose to
    # feature-major, cast bf16
    def prep(z):
        n = np.sqrt((z.astype(np.float64) ** 2).sum(axis=1, keepdims=True))
        zn = (z / np.maximum(n, EPS).astype(np.float32)).astype(np.float32)
        return np.ascontiguousarray(zn.T).astype(ml_dtypes.bfloat16)

    z1tn = prep(z1)  # [D, N] bf16
    z2tn = prep(z2)
    selw = _sel_weights()

    core_ids = list(range(NCORES))
    # strided row chunks: core c, group g -> rows [128*(8g+c), +128)
    in_maps = []
    for c in core_ids:
        cols = np.concatenate(
            [np.arange(128 * (8 * g + c), 128 * (8 * g + c) + 128) for g in range(G)]
        )
        in_maps.append(
            {
                "z1t": z1tn,
                "z2t": z2tn,
                "zb": np.ascontiguousarray(
                    np.concatenate([z1tn[:, cols], z2tn[:, cols]], axis=1)
                ),
                "sel": selw,
            }
        )

    nc = _get_nc()
    res = run_bass_kernel_spmd(
        nc,
        in_maps,
        core_ids,
        trace=bool(int(os.environ.get("KERNEL_TRACE", "0"))),
    )
    _cache["last_result"] = res

    # ---- host combine (the final all-reduce / mean) ----
    def gather_cs(name):
        v = np.zeros(N, dtype=np.float64)
        for c in core_ids:
            v += res.results[c][name].astype(np.float64).reshape(N)
        return v

    cs11_g = gather_cs("cs11")
    cs22_g = gather_cs("cs22")
    cs12_g = gather_cs("cs12")

    loss_sum = 0.0
    for c in core_ids:
        r = res.results[c]
        # local index l = g*128 + p  ->  global row 128*(8g+c) + p
        gl = np.concatenate(
            [np.arange(128 * (8 * g + c), 128 * (8 * g + c) + 128) for g in range(G)]
        )
        rs11 = r["rs11"].astype(np.float64).T.reshape(B)
        rs22 = r["rs22"].astype(np.float64).T.reshape(B)
        rs12 = r["rs12"].astype(np.float64).T.reshape(B)
        dg = r["diags"].astype(np.float64).reshape(3, B)
        d11, d12, d22 = dg[0], dg[1], dg[2]
        den1 = rs11 + cs11_g[gl] - np.exp(SCALE * d11) + rs12
        den2 = rs22 + cs22_g[gl] - np.exp(SCALE * d22) + cs12_g[gl]
        l = 0.5 * (np.log(den1) + np.log(den2)) - SCALE * d12
        loss_sum += l.sum()

    return np.float32(loss_sum / N)


# revision 25
# speedup vs baseline: 1.0104x; 1.0104x over previous
"""Contrastive loss (SimCLR-style semi_loss pair) on 8 Trainium2 NeuronCores.

Math (reference):
    z1n, z2n = L2-normalized rows of z1, z2            # [N, D], N=16384, D=128
    S11 = z1n @ z1n.T, S12 = z1n @ z2n.T, S22 = z2n @ z2n.T, S21 = S12.T
    d1_i = sum_j exp(2*S11_ij) - exp(2*S11_ii) + sum_j exp(2*S12_ij)
    d2_i = sum_j exp(2*S22_ij) - exp(2*S22_ii) + sum_j exp(2*S21_ij)
    loss = mean_i( 0.5*(log d1_i + log d2_i) - 2*S12_ii )

Device strategy (row-parallel over N, 8 cores, D=128 on the partition axis
so every Gram tile is one K=128 matmul):

* Every exponential is computed exactly once (ScalarE), on wide PSUM spans
  (2048/1536 alternating between two sim slots, 7 banks). Row sums ride
  the activation accumulator; exp values are also written to SBUF (bf16).
* Column sums (exp(2*S21) row sums; mirror halves of the symmetric refl
  matrices) use selector-weight matmuls: weights with ones in column j
  only, so the matmul adds the 512-wide column sum into row j of ONE
  shared PSUM accumulator bank and exact zeros everywhere else. All 32
  column chunks of a stream accumulate in that single bank (the 8th).
* S11/S22 are symmetric: only spans >= the 1024-aligned diagonal square
  are computed (row chunk g covers columns [1024g, 16384)); the lower
  triangle of each row sum is recovered from the column sums, with the
  diagonal square excluded from column sums to avoid double counting.
* SPMD: one NEFF for all 8 cores. Rows are sharded STRIDED (core c owns
  row chunks {128*(8g+c)}, g=0..15) so all cores share the same
  column-range structure. Host packs row blocks into `zb`, builds the
  selector weights, and does the final O(N) combine (column-sum
  all-reduce, logs, mean).
"""

import os

import numpy as np

N = 16384
D = 128
NCORES = 8
B = N // NCORES  # 2048 rows per core
TAU = 0.5
SCALE = 1.0 / TAU
EPS = 1e-12

G = 16  # row-chunk groups per core (128 rows each); chunk g -> cols >= 1024g
NCH = N // 512  # 32 column chunks of 512 (cs accumulator rows)

WA = 2048  # sim slot A width (4 banks)
WB = 1536  # sim slot B width (3 banks)


def _spans(l0, use_a=True):
    """Alternating A/B spans (last may be partial) covering [l0*512, N)."""
    spans = []
    off = l0 * 512
    while off < N:
        w = min(WA if use_a else WB, N - off)
        spans.append((off, w, use_a))
        use_a = not use_a
        off += w
    return spans, use_a


def _stream_plan(tri):
    """Per-chunk span lists with A/B alternation carried across chunks."""
    plan = []
    use_a = True
    for g in range(G):
        spans, use_a = _spans(2 * g if tri else 0, use_a)
        plan.append(spans)
    return plan


_cache = {}


def _build():
    from contextlib import ExitStack

    import concourse.mybir as mybir
    from concourse import bacc
    from concourse.tile import TileContext

    f32 = mybir.dt.float32
    bf16 = mybir.dt.bfloat16
    Exp = mybir.ActivationFunctionType.Exp
    add = mybir.AluOpType.add
    AX = mybir.AxisListType.X

    # Bacc (vs plain Bass) runs the wait-legalization passes at finalize:
    # move_matmul_waits_to_ldweights + generate_event_semaphores (TRN2 allows
    # at most one sync wait per hardware instruction).
    nc = bacc.Bacc(None, target_bir_lowering=False, name="contrastive_loss")

    z1t = nc.declare_dram_parameter("z1t", [D, N], bf16, isOutput=False)
    z2t = nc.declare_dram_parameter("z2t", [D, N], bf16, isOutput=False)
    # per-core strided row chunks: [z1 chunks g=0..15 | z2 chunks g=0..15]
    zb = nc.declare_dram_parameter("zb", [D, 2 * B], bf16, isOutput=False)
    # selector weights: sel[:, 128j + m] = 1 if m == j else 0 (j = 0..31)
    sel = nc.declare_dram_parameter("sel", [D, NCH * 128], bf16, isOutput=False)

    rs11_d = nc.declare_dram_parameter("rs11", [128, G], f32, isOutput=True)
    rs22_d = nc.declare_dram_parameter("rs22", [128, G], f32, isOutput=True)
    rs12_d = nc.declare_dram_parameter("rs12", [128, G], f32, isOutput=True)
    cs11_d = nc.declare_dram_parameter("cs11", [NCH, 512], f32, isOutput=True)
    cs22_d = nc.declare_dram_parameter("cs22", [NCH, 512], f32, isOutput=True)
    cs12_d = nc.declare_dram_parameter("cs12", [NCH, 512], f32, isOutput=True)
    # raw diagonal dot products: pairs (z1,z1),(z1,z2),(z2,z2), 4 chunks each
    dg_d = nc.declare_dram_parameter("diags", [12, 512], f32, isOutput=True)

    with TileContext(nc) as tc, ExitStack() as ctx:
        const = ctx.enter_context(tc.tile_pool(name="const", bufs=1))
        prodp = ctx.enter_context(tc.tile_pool(name="prodp", bufs=3))
        outp = ctx.enter_context(tc.tile_pool(name="outp", bufs=1))
        esbp = ctx.enter_context(tc.tile_pool(name="esbp", bufs=2))
        csout = ctx.enter_context(tc.tile_pool(name="csout", bufs=2))
        psS = ctx.enter_context(tc.tile_pool(name="psS", bufs=1, space="PSUM"))
        psCS = ctx.enter_context(tc.tile_pool(name="psCS", bufs=1, space="PSUM"))

        zb_sb = const.tile([128, 2 * B], bf16)
        sel_sb = const.tile([128, NCH * 128], bf16)
        z1t_sb = const.tile([128, N], bf16)
        z2t_sb = const.tile([128, N], bf16)
        nc.sync.dma_start(out=zb_sb, in_=zb[:, :])
        nc.sync.dma_start(out=sel_sb, in_=sel[:, :])
        ncol = N // 4
        for i in range(4):
            nc.sync.dma_start(
                out=z1t_sb[:, i * ncol : (i + 1) * ncol],
                in_=z1t[:, i * ncol : (i + 1) * ncol],
            )
        for i in range(4):
            nc.sync.dma_start(
                out=z2t_sb[:, i * ncol : (i + 1) * ncol],
                in_=z2t[:, i * ncol : (i + 1) * ncol],
            )
        z1b_sb = zb_sb[:, 0:B]
        z2b_sb = zb_sb[:, B : 2 * B]

        rs11_sb = outp.tile([128, G], f32, tag="rs11")
        rs22_sb = outp.tile([128, G], f32, tag="rs22")
        rs12_sb = outp.tile([128, G], f32, tag="rs12")
        parts11 = outp.tile([128, G * 16], f32, tag="p11")
        parts22 = outp.tile([128, G * 16], f32, tag="p22")
        parts12 = outp.tile([128, G * 16], f32, tag="p12")

        def wsel(j):
            return sel_sb[:, j * 128 : (j + 1) * 128]

        # ---- Phase 0: raw diagonals diag[i] = sum_d a[d,i]*b[d,i] ----
        # elementwise products, then selector-matmul column sums into one
        # accumulator bank (rows 0..11), evacuated once.
        dgps = psCS.tile([128, 512], f32, tag="cs", name="dgps")
        pairs = [(z1b_sb, z1b_sb), (z1b_sb, z2b_sb), (z2b_sb, z2b_sb)]
        first = True
        for di, (a, b) in enumerate(pairs):
            prod = prodp.tile([128, B], bf16)
            nc.vector.tensor_mul(prod, a, b)
            for k in range(4):
                nc.tensor.matmul(
                    dgps,
                    lhsT=wsel(4 * di + k),
                    rhs=prod[:, k * 512 : (k + 1) * 512],
                    start=first,
                    stop=(di == 2 and k == 3),
                )
                first = False
        dg_sb = csout.tile([12, 512], f32, tag="dg")
        nc.vector.tensor_copy(out=dg_sb, in_=dgps[0:12, :])
        nc.sync.dma_start(out=dg_d[:, :], in_=dg_sb)

        # ---- main streams ----
        # (zoff, rhs, tri, parts, cs_d, rs_sb, rs_d)
        streams = [
            (0, z1t_sb, True, parts11, cs11_d, rs11_sb, rs11_d),
            (B, z2t_sb, True, parts22, cs22_d, rs22_sb, rs22_d),
            (0, z2t_sb, False, parts12, cs12_d, rs12_sb, rs12_d),
        ]
        for si, (zoff, full, tri, parts, cs_d_, rs_sb, rs_d) in enumerate(streams):
            plan = _stream_plan(tri)
            # enumerate the column-sum matmuls up front so start/stop flags
            # land on the stream's true first/last ones
            n_ones = sum(
                1
                for g in range(G)
                for (off, width, _a) in plan[g]
                for k in range(width // 512)
                if not (tri and (off + k * 512) // 512 in (2 * g, 2 * g + 1))
            )
            csps = psCS.tile([128, 512], f32, tag="cs", name=f"csps{si}")
            ones_idx = 0
            pending = []  # previous span's column-sum matmuls (esb, j) pairs

            def flush_pending():
                nonlocal ones_idx, pending
                for esb_, k_, j_ in pending:
                    nc.tensor.matmul(
                        csps,
                        lhsT=wsel(j_),
                        rhs=esb_[:, k_ * 512 : (k_ + 1) * 512],
                        start=(ones_idx == 0),
                        stop=(ones_idx == n_ones - 1),
                    )
                    ones_idx += 1
                pending = []

            for g in range(G):
                for sp, (off, width, use_a) in enumerate(plan[g]):
                    nk = width // 512
                    sim = psS.tile(
                        [128, WA if use_a else WB],
                        f32,
                        tag="simA" if use_a else "simB",
                        name="simA_t" if use_a else "simB_t",
                    )
                    for k in range(nk):
                        col = off + k * 512
                        nc.tensor.matmul(
                            sim[:, k * 512 : (k + 1) * 512],
                            lhsT=zb_sb[:, zoff + g * 128 : zoff + (g + 1) * 128],
                            rhs=full[:, col : col + 512],
                            start=True,
                            stop=True,
                        )
                    esb = esbp.tile(
                        [128, WA if use_a else WB],
                        bf16,
                        tag="esbA" if use_a else "esbB",
                        name="esbA_t" if use_a else "esbB_t",
                    )
                    nc.scalar.activation(
                        out=esb[:, 0:width],
                        in_=sim[:, 0:width],
                        func=Exp,
                        scale=SCALE,
                        accum_out=parts[:, g * 16 + sp : g * 16 + sp + 1],
                    )
                    # column-sum matmuls for the PREVIOUS span run while this
                    # span's ACTIVATE executes (PE is in-order; emitting them
                    # here would stall the next span's sims on this ACT)
                    flush_pending()
                    for k in range(nk):
                        j = (off + k * 512) // 512
                        if tri and j in (2 * g, 2 * g + 1):
                            continue
                        pending.append((esb, k, j))
            flush_pending()
            cs_sb = csout.tile([NCH, 512], f32, tag="cs", name=f"cs_sb{si}")
            nc.vector.tensor_copy(out=cs_sb, in_=csps[0:NCH, :])
            nc.sync.dma_start(out=cs_d_[:, :], in_=cs_sb)

            # row sums: reduce the per-span partials
            for g in range(G):
                nsp = len(plan[g])
                nc.vector.tensor_reduce(
                    out=rs_sb[:, g : g + 1],
                    in_=parts[:, g * 16 : g * 16 + nsp],
                    axis=AX,
                    op=add,
                )
            nc.sync.dma_start(out=rs_d[:, :], in_=rs_sb)

    nc.finalize()  # Bacc: runs wait-legalization + register allocation
    return nc


def _get_nc():
    if "nc" not in _cache:
        _cache["nc"] = _build()
    return _cache["nc"]


def _sel_weights():
    import ml_dtypes

    w = np.zeros((D, NCH, 128), dtype=np.float32)
    for j in range(NCH):
        w[:, j, j] = 1.0
    return np.ascontiguousarray(w.reshape(D, NCH * 128)).astype(ml_dtypes.bfloat16)


def kernel(z1: np.ndarray, z2: np.ndarray) -> np.ndarray:
    import ml_dtypes

    from concourse.bass_utils import run_bass_kernel_spmd

    z1 = np.asarray(z1, dtype=np.float32)
    z2 = np.asarray(z2, dtype=np.float32)

    # host: L2 row-normalize (matches F.normalize eps clamp), transpose to
    # feature-major, cast bf16
    def prep(z):
        n = np.sqrt((z.astype(np.float64) ** 2).sum(axis=1, keepdims=True))
        zn = (z / np.maximum(n, EPS).astype(np.float32)).astype(np.float32)
        return np.ascontiguousarray(zn.T).astype(ml_dtypes.bfloat16)

    z1tn = prep(z1)  # [D, N] bf16
    z2tn = prep(z2)
    selw = _sel_weights()

    core_ids = list(range(NCORES))
    # strided row chunks: core c, group g -> rows [128*(8g+c), +128)
    in_maps = []
    for c in core_ids:
        cols = np.concatenate(
            [np.arange(128 * (8 * g + c), 128 * (8 * g + c) + 128) for g in range(G)]
        )
        in_maps.append(
            {
                "z1t": z1tn,
                "z2t": z2tn,
                "zb": np.ascontiguousarray(
                    np.concatenate([z1tn[:, cols], z2tn[:, cols]], axis=1)
                ),
                "sel": selw,
            }
        )

    nc = _get_nc()
    res = run_bass_kernel_spmd(
        nc,
        in_maps,
        core_ids,
        trace=bool(int(os.environ.get("KERNEL_TRACE", "0"))),
    )
    _cache["last_result"] = res

    # ---- host combine (the final all-reduce / mean) ----
    def gather_cs(name):
        v = np.zeros(N, dtype=np.float64)
        for c in core_ids:
            v += res.results[c][name].astype(np.float64).reshape(N)
        return v

    cs11_g = gather_cs("cs11")
    cs22_g = gather_cs("cs22")
    cs12_g = gather_cs("cs12")

    loss_sum = 0.0
    for c in core_ids:
        r = res.results[c]
        # local index l = g*128 + p  ->  global row 128*(8g+c) + p
        gl = np.concatenate(
            [np.arange(128 * (8 * g + c), 128 * (8 * g + c) + 128) for g in range(G)]
        )
        rs11 = r["rs11"].astype(np.float64).T.reshape(B)
        rs22 = r["rs22"].astype(np.float64).T.reshape(B)
        rs12 = r["rs12"].astype(np.float64).T.reshape(B)
        dg = r["diags"].astype(np.float64).reshape(3, B)
        d11, d12, d22 = dg[0], dg[1], dg[2]
        den1 = rs11 + cs11_g[gl] - np.exp(SCALE * d11) + rs12
        den2 = rs22 + cs22_g[gl] - np.exp(SCALE * d22) + cs12_g[gl]
        l = 0.5 * (np.log(den1) + np.log(den2)) - SCALE * d12
        loss_sum += l.sum()

    return np.float32(loss_sum / N)


# revision 31
# speedup vs baseline: 1.1297x; 1.1180x over previous
"""Contrastive loss (SimCLR-style semi_loss pair) on 8 Trainium2 NeuronCores.

Math (reference):
    z1n, z2n = L2-normalized rows of z1, z2            # [N, D], N=16384, D=128
    S11 = z1n @ z1n.T, S12 = z1n @ z2n.T, S22 = z2n @ z2n.T, S21 = S12.T
    d1_i = sum_j exp(2*S11_ij) - exp(2*S11_ii) + sum_j exp(2*S12_ij)
    d2_i = sum_j exp(2*S22_ij) - exp(2*S22_ii) + sum_j exp(2*S21_ij)
    loss = mean_i( 0.5*(log d1_i + log d2_i) - 2*S12_ii )

Device strategy (row-parallel over N, 8 cores, D=128 on the partition axis
so every Gram tile is one K=128 matmul):

* Every exponential is computed exactly once (ScalarE), on wide PSUM spans
  (2048/1536 alternating between two sim slots, 7 banks). Row sums ride
  the activation accumulator; exp values are also written to SBUF (bf16).
* Column sums (exp(2*S21) row sums; mirror halves of the symmetric refl
  matrices) use selector-weight matmuls: weights with ones in column j
  only, so the matmul adds the 512-wide column sum into row j of ONE
  shared PSUM accumulator bank and exact zeros everywhere else. All 32
  column chunks of a stream accumulate in that single bank (the 8th).
* S11/S22 are symmetric: only spans >= the 1024-aligned diagonal square
  are computed (row chunk g covers columns [1024g, 16384)); the lower
  triangle of each row sum is recovered from the column sums, with the
  diagonal square excluded from column sums to avoid double counting.
* SPMD: one NEFF for all 8 cores. Rows are sharded STRIDED (core c owns
  row chunks {128*(8g+c)}, g=0..15) so all cores share the same
  column-range structure. Host packs row blocks into `zb`, builds the
  selector weights, and does the final O(N) combine (column-sum
  all-reduce, logs, mean).
"""

import os

import numpy as np

N = 16384
D = 128
NCORES = 8
B = N // NCORES  # 2048 rows per core
TAU = 0.5
SCALE = 1.0 / TAU
EPS = 1e-12

G = 16  # row-chunk groups per core (128 rows each); chunk g -> cols >= 1024g
NCH = N // 512  # 32 column chunks of 512 (cs accumulator rows)

WA = 2048  # sim slot A width (4 banks)
WB = 1536  # sim slot B width (3 banks)


def _spans(l0, use_a=True):
    """Alternating A/B spans (last may be partial) covering [l0*512, N)."""
    spans = []
    off = l0 * 512
    while off < N:
        w = min(WA if use_a else WB, N - off)
        spans.append((off, w, use_a))
        use_a = not use_a
        off += w
    return spans, use_a


def _stream_plan(tri):
    """Per-chunk span lists with A/B alternation carried across chunks."""
    plan = []
    use_a = True
    for g in range(G):
        spans, use_a = _spans(2 * g if tri else 0, use_a)
        plan.append(spans)
    return plan


_cache = {}


def _build():
    from contextlib import ExitStack

    import concourse.mybir as mybir
    from concourse import bacc
    from concourse.tile import TileContext

    f32 = mybir.dt.float32
    bf16 = mybir.dt.bfloat16
    Exp = mybir.ActivationFunctionType.Exp
    add = mybir.AluOpType.add
    AX = mybir.AxisListType.X

    # Bacc (vs plain Bass) runs the wait-legalization passes at finalize:
    # move_matmul_waits_to_ldweights + generate_event_semaphores (TRN2 allows
    # at most one sync wait per hardware instruction).
    nc = bacc.Bacc(None, target_bir_lowering=False, name="contrastive_loss")

    z1t = nc.declare_dram_parameter("z1t", [D, N], bf16, isOutput=False)
    z2t = nc.declare_dram_parameter("z2t", [D, N], bf16, isOutput=False)
    # per-core strided row chunks: [z1 chunks g=0..15 | z2 chunks g=0..15]
    zb = nc.declare_dram_parameter("zb", [D, 2 * B], bf16, isOutput=False)
    # selector weights: sel[:, 128j + m] = 1 if m == j else 0 (j = 0..31)
    sel = nc.declare_dram_parameter("sel", [D, NCH * 128], bf16, isOutput=False)

    rs11_d = nc.declare_dram_parameter("rs11", [128, G], f32, isOutput=True)
    rs22_d = nc.declare_dram_parameter("rs22", [128, G], f32, isOutput=True)
    rs12_d = nc.declare_dram_parameter("rs12", [128, G], f32, isOutput=True)
    cs11_d = nc.declare_dram_parameter("cs11", [NCH, 512], f32, isOutput=True)
    cs22_d = nc.declare_dram_parameter("cs22", [NCH, 512], f32, isOutput=True)
    cs12_d = nc.declare_dram_parameter("cs12", [NCH, 512], f32, isOutput=True)
    # raw diagonal dot products: pairs (z1,z1),(z1,z2),(z2,z2), 4 chunks each
    dg_d = nc.declare_dram_parameter("diags", [12, 512], f32, isOutput=True)

    with TileContext(nc) as tc, ExitStack() as ctx:
        const = ctx.enter_context(tc.tile_pool(name="const", bufs=1))
        prodp = ctx.enter_context(tc.tile_pool(name="prodp", bufs=3))
        outp = ctx.enter_context(tc.tile_pool(name="outp", bufs=1))
        esbp = ctx.enter_context(tc.tile_pool(name="esbp", bufs=3))
        csout = ctx.enter_context(tc.tile_pool(name="csout", bufs=2))
        psS = ctx.enter_context(tc.tile_pool(name="psS", bufs=1, space="PSUM"))
        psCS = ctx.enter_context(tc.tile_pool(name="psCS", bufs=1, space="PSUM"))

        zb_sb = const.tile([128, 2 * B], bf16)
        sel_sb = const.tile([128, NCH * 128], bf16)
        z1t_sb = const.tile([128, N], bf16)
        z2t_sb = const.tile([128, N], bf16)
        nc.sync.dma_start(out=zb_sb, in_=zb[:, :])
        nc.sync.dma_start(out=sel_sb, in_=sel[:, :])
        ncol = N // 8
        for i in range(8):
            nc.sync.dma_start(
                out=z1t_sb[:, i * ncol : (i + 1) * ncol],
                in_=z1t[:, i * ncol : (i + 1) * ncol],
            )
        ncol = N // 4
        for i in range(4):
            nc.sync.dma_start(
                out=z2t_sb[:, i * ncol : (i + 1) * ncol],
                in_=z2t[:, i * ncol : (i + 1) * ncol],
            )
        z1b_sb = zb_sb[:, 0:B]
        z2b_sb = zb_sb[:, B : 2 * B]

        rs11_sb = outp.tile([128, G], f32, tag="rs11")
        rs22_sb = outp.tile([128, G], f32, tag="rs22")
        rs12_sb = outp.tile([128, G], f32, tag="rs12")
        parts11 = outp.tile([128, G * 16], f32, tag="p11")
        parts22 = outp.tile([128, G * 16], f32, tag="p22")
        parts12 = outp.tile([128, G * 16], f32, tag="p12")

        def wsel(j):
            return sel_sb[:, j * 128 : (j + 1) * 128]

        # ---- main streams ----
        # (zoff, rhs, tri, parts, cs_d, rs_sb, rs_d)
        streams = [
            (0, z1t_sb, True, parts11, cs11_d, rs11_sb, rs11_d),
            (B, z2t_sb, True, parts22, cs22_d, rs22_sb, rs22_d),
            (0, z2t_sb, False, parts12, cs12_d, rs12_sb, rs12_d),
        ]
        for si, (zoff, full, tri, parts, cs_d_, rs_sb, rs_d) in enumerate(streams):
            plan = _stream_plan(tri)
            # enumerate the column-sum matmuls up front so start/stop flags
            # land on the stream's true first/last ones
            n_ones = sum(
                1
                for g in range(G)
                for (off, width, _a) in plan[g]
                for k in range(width // 512)
                if not (tri and (off + k * 512) // 512 in (2 * g, 2 * g + 1))
            )
            csps = psCS.tile([128, 512], f32, tag="cs", name=f"csps{si}")
            ones_idx = 0
            pending = []  # deferred column-sum matmuls [(esb, k, j), ...]

            def flush_pending(keep=0):
                nonlocal ones_idx, pending
                while len(pending) > keep:
                    esb_, k_, j_ = pending.pop(0)
                    nc.tensor.matmul(
                        csps,
                        lhsT=wsel(j_),
                        rhs=esb_[:, k_ * 512 : (k_ + 1) * 512],
                        start=(ones_idx == 0),
                        stop=(ones_idx == n_ones - 1),
                    )
                    ones_idx += 1

            for g in range(G):
                for sp, (off, width, use_a) in enumerate(plan[g]):
                    nk = width // 512
                    sim = psS.tile(
                        [128, WA if use_a else WB],
                        f32,
                        tag="simA" if use_a else "simB",
                        name="simA_t" if use_a else "simB_t",
                    )
                    for k in range(nk):
                        col = off + k * 512
                        nc.tensor.matmul(
                            sim[:, k * 512 : (k + 1) * 512],
                            lhsT=zb_sb[:, zoff + g * 128 : zoff + (g + 1) * 128],
                            rhs=full[:, col : col + 512],
                            start=True,
                            stop=True,
                        )
                    esb = esbp.tile(
                        [128, WA if use_a else WB],
                        bf16,
                        tag="esbA" if use_a else "esbB",
                        name="esbA_t" if use_a else "esbB_t",
                    )
                    nc.scalar.activation(
                        out=esb[:, 0:width],
                        in_=sim[:, 0:width],
                        func=Exp,
                        scale=SCALE,
                        accum_out=parts[:, g * 16 + sp : g * 16 + sp + 1],
                    )
                    # deferred column-sum matmuls run while later spans'
                    # ACTIVATEs execute (PE is in-order; emitting them here
                    # would stall the next span's sims on this ACT)
                    nxt = [
                        (esb, k, (off + k * 512) // 512)
                        for k in range(nk)
                        if not (tri and (off + k * 512) // 512 in (2 * g, 2 * g + 1))
                    ]
                    flush_pending(keep=max(0, 8 - len(nxt)))
                    pending.extend(nxt)
            flush_pending()
            cs_sb = csout.tile([NCH, 512], f32, tag="cs", name=f"cs_sb{si}")
            nc.vector.tensor_copy(out=cs_sb, in_=csps[0:NCH, :])
            nc.sync.dma_start(out=cs_d_[:, :], in_=cs_sb)

            # row sums: reduce the per-span partials
            for g in range(G):
                nsp = len(plan[g])
                nc.vector.tensor_reduce(
                    out=rs_sb[:, g : g + 1],
                    in_=parts[:, g * 16 : g * 16 + nsp],
                    axis=AX,
                    op=add,
                )
            nc.sync.dma_start(out=rs_d[:, :], in_=rs_sb)

        # ---- Phase 0 (emitted last; independent of the streams): raw
        # diagonals diag[i] = sum_d a[d,i]*b[d,i] via elementwise products +
        # selector-matmul column sums into the accumulator bank (rows 0..11)
        dgps = psCS.tile([128, 512], f32, tag="cs", name="dgps")
        pairs = [(z1b_sb, z1b_sb), (z1b_sb, z2b_sb), (z2b_sb, z2b_sb)]
        first = True
        for di, (a, b) in enumerate(pairs):
            prod = prodp.tile([128, B], bf16)
            nc.vector.tensor_mul(prod, a, b)
            for k in range(4):
                nc.tensor.matmul(
                    dgps,
                    lhsT=wsel(4 * di + k),
                    rhs=prod[:, k * 512 : (k + 1) * 512],
                    start=first,
                    stop=(di == 2 and k == 3),
                )
                first = False
        dg_sb = csout.tile([12, 512], f32, tag="dg")
        nc.vector.tensor_copy(out=dg_sb, in_=dgps[0:12, :])
        nc.sync.dma_start(out=dg_d[:, :], in_=dg_sb)

    nc.finalize()  # Bacc: runs wait-legalization + register allocation
    return nc


def _get_nc():
    if "nc" not in _cache:
        _cache["nc"] = _build()
    return _cache["nc"]


def _sel_weights():
    import ml_dtypes

    w = np.zeros((D, NCH, 128), dtype=np.float32)
    for j in range(NCH):
        w[:, j, j] = 1.0
    return np.ascontiguousarray(w.reshape(D, NCH * 128)).astype(ml_dtypes.bfloat16)


def kernel(z1: np.ndarray, z2: np.ndarray) -> np.ndarray:
    import ml_dtypes

    from concourse.bass_utils import run_bass_kernel_spmd

    z1 = np.asarray(z1, dtype=np.float32)
    z2 = np.asarray(z2, dtype=np.float32)

    # host: L2 row-normalize (matches F.normalize eps clamp), transpose to
    # feature-major, cast bf16
    def prep(z):
        n = np.sqrt((z.astype(np.float64) ** 2).sum(axis=1, keepdims=True))
        zn = (z / np.maximum(n, EPS).astype(np.float32)).astype(np.float32)
        return np.ascontiguousarray(zn.T).astype(ml_dtypes.bfloat16)

    z1tn = prep(z1)  # [D, N] bf16
    z2tn = prep(z2)
    selw = _sel_weights()

    core_ids = list(range(NCORES))
    # strided row chunks: core c, group g -> rows [128*(8g+c), +128)
    in_maps = []
    for c in core_ids:
        cols = np.concatenate(
            [np.arange(128 * (8 * g + c), 128 * (8 * g + c) + 128) for g in range(G)]
        )
        in_maps.append(
            {
                "z1t": z1tn,
                "z2t": z2tn,
                "zb": np.ascontiguousarray(
                    np.concatenate([z1tn[:, cols], z2tn[:, cols]], axis=1)
                ),
                "sel": selw,
            }
        )

    nc = _get_nc()
    res = run_bass_kernel_spmd(
        nc,
        in_maps,
        core_ids,
        trace=bool(int(os.environ.get("KERNEL_TRACE", "0"))),
    )
    _cache["last_result"] = res

    # ---- host combine (the final all-reduce / mean) ----
    def gather_cs(name):
        v = np.zeros(N, dtype=np.float64)
        for c in core_ids:
            v += res.results[c][name].astype(np.float64).reshape(N)
        return v

    cs11_g = gather_cs("cs11")
    cs22_g = gather_cs("cs22")
    cs12_g = gather_cs("cs12")

    loss_sum = 0.0
    for c in core_ids:
        r = res.results[c]
        # local index l = g*128 + p  ->  global row 128*(8g+c) + p
        gl = np.concatenate(
            [np.arange(128 * (8 * g + c), 128 * (8 * g + c) + 128) for g in range(G)]
        )
        rs11 = r["rs11"].astype(np.float64).T.reshape(B)
        rs22 = r["rs22"].astype(np.float64).T.reshape(B)
        rs12 = r["rs12"].astype(np.float64).T.reshape(B)
        dg = r["diags"].astype(np.float64).reshape(3, B)
        d11, d12, d22 = dg[0], dg[1], dg[2]
        den1 = rs11 + cs11_g[gl] - np.exp(SCALE * d11) + rs12
        den2 = rs22 + cs22_g[gl] - np.exp(SCALE * d22) + cs12_g[gl]
        l = 0.5 * (np.log(den1) + np.log(den2)) - SCALE * d12
        loss_sum += l.sum()

    return np.float32(loss_sum / N)
